# revision 7
# baseline (speedup 1.0000x reference)
"""BMN extractor kernel for Trainium2 (8 NeuronCores, Bass/Tile).

Computation (matches the reference nn.Module):
  h   = relu(conv1d(x, w_red, k=3, pad=SAME) + b_red)            [B, CH, T]
  map = einsum('bct,tndm->bcndm', h, mask)                        (never materialized)
  m3  = relu(einsum('ocn,bcndm->bodm', w3d, map) + b3d)           [B, CR, D, M]
  out = relu(einsum('oc,bcdm->bodm', w2d, m3) + b2d)              [B, CO, D, M]

Reassociation used on device:
  P[b,o,n,t]  = sum_c w3d[o,c,n] * h[b,c,t]            (small matmuls)
  m3[b,o,d,m] = sum_{n,t} P[b,o,n,t] * mask[t,n,d,m]   (big matmul, K=N*T=4096)

Sharding: dscale (D=128) split across the 8 cores (16 durations each); every
core computes all of h/P, reads only its own 1/8 slice of the mask.
"""

import os

import numpy as np
import ml_dtypes

B, C_IN, C_HID, C_ROI, C_OUT = 2, 256, 128, 512, 128
T, N, D, M = 128, 32, 128, 128
NCORES = 8
DD = D // NCORES          # durations per core
DG = 4                    # durations per dm-tile (DG*M = 512 psum columns)
NJ = DD // DG             # dm-tiles per core
BF = ml_dtypes.bfloat16

_CACHE = {}
LAST_EXEC_NS = None


def _build():
    import concourse.tile as tile
    from concourse import bacc, mybir

    bf16 = mybir.dt.bfloat16
    f32 = mybir.dt.float32
    Relu = mybir.ActivationFunctionType.Relu

    nc = bacc.Bacc(None, target_bir_lowering=False)
    x_d = nc.dram_tensor("x_bf", [B, C_IN, T + 2], bf16, kind="ExternalInput")
    wred_d = nc.dram_tensor("wred_t", [3, C_IN, C_HID], bf16, kind="ExternalInput")
    bred_d = nc.dram_tensor("bred", [C_HID, 1], f32, kind="ExternalInput")
    w3d_d = nc.dram_tensor("w3d_t", [N, C_HID, C_ROI], bf16, kind="ExternalInput")
    b3d_d = nc.dram_tensor("b3d", [C_ROI, 1], f32, kind="ExternalInput")
    w2d_d = nc.dram_tensor("w2d_t", [C_ROI, C_OUT], bf16, kind="ExternalInput")
    b2d_d = nc.dram_tensor("b2d", [C_OUT, 1], f32, kind="ExternalInput")
    mask_d = nc.dram_tensor("mask", [T, N * DD * M], bf16, kind="ExternalInput")
    out_d = nc.dram_tensor("out", [B, C_OUT, DD, M], f32, kind="ExternalOutput")

    mask_v = mask_d.rearrange("t (n d m) -> t n d m", n=N, d=DD, m=M)

    with tile.TileContext(nc) as tc:
        with (
            tc.tile_pool(name="consts", bufs=1) as consts,
            tc.tile_pool(name="xpool", bufs=1) as xpool,
            tc.tile_pool(name="hpool", bufs=1) as hpool,
            tc.tile_pool(name="w3pool", bufs=1) as w3pool,
            tc.tile_pool(name="ppool", bufs=1) as ppool,
            tc.tile_pool(name="maskpool", bufs=2) as maskpool,
            tc.tile_pool(name="m3pool", bufs=2) as m3pool,
            tc.tile_pool(name="outpool", bufs=4) as outpool,
            tc.tile_pool(name="ps_a", bufs=1, space="PSUM") as ps_a,
            tc.tile_pool(name="ps_b", bufs=2, space="PSUM") as ps_b,
            tc.tile_pool(name="ps_c", bufs=3, space="PSUM") as ps_c,
            tc.tile_pool(name="ps_d", bufs=2, space="PSUM") as ps_d,
        ):
            # ---------------- constants ----------------
            wred_sb = consts.tile([128, 6 * C_HID], bf16)
            for k in range(3):
                for u in range(2):
                    nc.sync.dma_start(
                        wred_sb[:, (k * 2 + u) * C_HID:(k * 2 + u + 1) * C_HID],
                        wred_d[k, u * 128:(u + 1) * 128, :],
                    )
            bred_sb = consts.tile([C_HID, 1], f32)
            nc.sync.dma_start(bred_sb[:], bred_d[:, :])
            b3d_sb = consts.tile([128, 4], f32)
            for g in range(4):
                nc.sync.dma_start(b3d_sb[:, g:g + 1], b3d_d[g * 128:(g + 1) * 128, :])
            b2d_sb = consts.tile([C_OUT, 1], f32)
            nc.sync.dma_start(b2d_sb[:], b2d_d[:, :])
            w2d_sb = consts.tile([128, 4 * C_OUT], bf16)
            for g in range(4):
                nc.sync.dma_start(
                    w2d_sb[:, g * C_OUT:(g + 1) * C_OUT],
                    w2d_d[g * 128:(g + 1) * 128, :],
                )
            w3d_sb = w3pool.tile([C_HID, N * C_ROI], bf16)
            nc.sync.dma_start(w3d_sb[:], w3d_d[:, :, :].rearrange("n c o -> c n o"))

            # ---------------- stage A: conv1d + relu -> h ----------------
            h_sb = []
            for b in range(B):
                xp = []
                for u in range(2):
                    xt = xpool.tile([128, T + 2], bf16, tag=f"x_{b}_{u}", name=f"x_{b}_{u}")
                    nc.sync.dma_start(xt[:], x_d[b, u * 128:(u + 1) * 128, :])
                    xp.append(xt)
                hp = ps_a.tile([C_HID, T], f32, tag="hps", name=f"hps_{b}")
                first = True
                for u in range(2):
                    for k in range(3):
                        nc.tensor.matmul(
                            hp[:],
                            wred_sb[:, (k * 2 + u) * C_HID:(k * 2 + u + 1) * C_HID],
                            xp[u][:, k:k + T],
                            start=first,
                            stop=(u == 1 and k == 2),
                        )
                        first = False
                ht = hpool.tile([C_HID, T], bf16, tag=f"h_{b}", name=f"h_{b}")
                nc.scalar.activation(ht[:], hp[:], Relu, bias=bred_sb[:, 0:1])
                h_sb.append(ht)

            # ---------------- stage B: P^T[b,n] = [t, o] ----------------
            P = [[None] * N for _ in range(B)]
            cnt = 0
            for b in range(B):
                for n in range(N):
                    pp = ps_b.tile([T, C_ROI], f32, tag="pps", name=f"pps_{b}_{n}")
                    nc.tensor.matmul(
                        pp[:], h_sb[b][:], w3d_sb[:, n * C_ROI:(n + 1) * C_ROI],
                        start=True, stop=True,
                    )
                    pt = ppool.tile([T, C_ROI], bf16, tag=f"P_{b}_{n}", name=f"P_{b}_{n}")
                    if cnt % 2 == 0:
                        nc.vector.tensor_copy(pt[:], pp[:])
                    else:
                        nc.scalar.copy(pt[:], pp[:])
                    cnt += 1
                    P[b][n] = pt

            # ---------------- stages C & D per dm-tile ----------------
            for j in range(NJ):
                mt = maskpool.tile([T, N * DG * M], bf16, tag="mask", name=f"mask_{j}")
                nc.sync.dma_start(mt[:], mask_v[:, :, j * DG:(j + 1) * DG, :])
                m3 = [[None] * 4 for _ in range(B)]
                for b in range(B):
                    for o4 in range(4):
                        pc = ps_c.tile([128, DG * M], f32, tag="m3ps", name=f"m3ps_{j}_{b}_{o4}")
                        for n in range(N):
                            nc.tensor.matmul(
                                pc[:],
                                P[b][n][:, o4 * 128:(o4 + 1) * 128],
                                mt[:, n * DG * M:(n + 1) * DG * M],
                                start=(n == 0),
                                stop=(n == N - 1),
                            )
                        m3t = m3pool.tile([128, DG * M], bf16, tag=f"m3_{b}_{o4}", name=f"m3_{j}_{b}_{o4}")
                        nc.scalar.activation(m3t[:], pc[:], Relu, bias=b3d_sb[:, o4:o4 + 1])
                        m3[b][o4] = m3t
                for b in range(B):
                    pd = ps_d.tile([C_OUT, DG * M], f32, tag="outps", name=f"outps_{j}_{b}")
                    for o4 in range(4):
                        nc.tensor.matmul(
                            pd[:],
                            w2d_sb[:, o4 * C_OUT:(o4 + 1) * C_OUT],
                            m3[b][o4][:],
                            start=(o4 == 0),
                            stop=(o4 == 3),
                        )
                    ot = outpool.tile([C_OUT, DG * M], f32, tag="out", name=f"out_{j}_{b}")
                    nc.scalar.activation(ot[:], pd[:], Relu, bias=b2d_sb[:, 0:1])
                    nc.sync.dma_start(out_d[b, :, j * DG:(j + 1) * DG, :], ot[:])
    nc.compile()
    return nc


def kernel(**inputs):
    global LAST_EXEC_NS
    x = np.asarray(inputs["x"], dtype=np.float32)
    w_red = np.asarray(inputs["w_red"], dtype=np.float32)
    b_red = np.asarray(inputs["b_red"], dtype=np.float32)
    w3d = np.asarray(inputs["w3d"], dtype=np.float32)
    b3d = np.asarray(inputs["b3d"], dtype=np.float32)
    w2d = np.asarray(inputs["w2d"], dtype=np.float32)
    b2d = np.asarray(inputs["b2d"], dtype=np.float32)
    mask = np.asarray(inputs["sample_mask"], dtype=np.float32)

    x_bf = np.zeros((B, C_IN, T + 2), dtype=BF)
    x_bf[:, :, 1:T + 1] = x.astype(BF)
    wred_t = np.ascontiguousarray(w_red.transpose(2, 1, 0)).astype(BF)   # [3, CI, CH]
    w3d_t = np.ascontiguousarray(w3d.transpose(2, 1, 0)).astype(BF)      # [N, CH, CR]
    w2d_t = np.ascontiguousarray(w2d.transpose(1, 0)).astype(BF)         # [CR, CO]
    bred = np.ascontiguousarray(b_red.reshape(C_HID, 1))
    b3d_r = np.ascontiguousarray(b3d.reshape(C_ROI, 1))
    b2d_r = np.ascontiguousarray(b2d.reshape(C_OUT, 1))
    mask_bf = mask.astype(BF)                                            # [T, N, D, M]

    common = dict(x_bf=x_bf, wred_t=wred_t, bred=bred, w3d_t=w3d_t,
                  b3d=b3d_r, w2d_t=w2d_t, b2d=b2d_r)
    in_maps = []
    for c in range(NCORES):
        mk = np.ascontiguousarray(mask_bf[:, :, c * DD:(c + 1) * DD, :])
        in_maps.append(dict(common, mask=mk.reshape(T, N * DD * M)))

    if "nc" not in _CACHE:
        _CACHE["nc"] = _build()
    nc = _CACHE["nc"]

    from concourse.bass_utils import run_bass_kernel_spmd

    trace = os.environ.get("BMN_TRACE", "0") == "1"
    res = run_bass_kernel_spmd(nc, in_maps, core_ids=list(range(NCORES)), trace=trace)
    LAST_EXEC_NS = res.exec_time_ns
    out = np.concatenate([r["out"] for r in res.results], axis=2)
    return out


# revision 8
# speedup vs baseline: 1.3767x; 1.3767x over previous
"""BMN extractor kernel for Trainium2 (8 NeuronCores, Bass/Tile).

Computation (matches the reference nn.Module):
  h   = relu(conv1d(x, w_red, k=3, pad=SAME) + b_red)            [B, CH, T]
  map = einsum('bct,tndm->bcndm', h, mask)                        (never materialized)
  m3  = relu(einsum('ocn,bcndm->bodm', w3d, map) + b3d)           [B, CR, D, M]
  out = relu(einsum('oc,bcdm->bodm', w2d, m3) + b2d)              [B, CO, D, M]

Reassociation used on device:
  P[b,o,n,t]  = sum_c w3d[o,c,n] * h[b,c,t]            (small matmuls)
  m3[b,o,d,m] = sum_{n,t} P[b,o,n,t] * mask[t,n,d,m]   (big matmul, K=N*T=4096)

Cells with d+m >= T have an all-zero mask column, so their output is a
per-channel constant relu(w2d @ relu(b3d) + b2d) — computed host-side.  Only
the 50.4% valid columns are computed on device.  Durations are sharded across
the 8 cores in pairs (d, 127-d) so every core gets exactly 1032 valid
(d,m) columns, padded to W=1088.
"""

import os

import numpy as np
import ml_dtypes

B, C_IN, C_HID, C_ROI, C_OUT = 2, 256, 128, 512, 128
T, N, D, M = 128, 32, 128, 128
NCORES = 8
W = 1088                       # padded packed (d,m) columns per core
COL_TILES = [(0, 512), (512, 512), (1024, 64)]
BF = ml_dtypes.bfloat16

_CACHE = {}
LAST_EXEC_NS = None


def _dlist(core):
    """Duration values handled by `core`: 8 pairs (i, 127-i) -> 1032 valid cols."""
    out = []
    for i in range(core, 64, 8):
        out += [i, 127 - i]
    return out


def _build():
    import concourse.tile as tile
    from concourse import bacc, mybir

    bf16 = mybir.dt.bfloat16
    f32 = mybir.dt.float32
    Relu = mybir.ActivationFunctionType.Relu

    nc = bacc.Bacc(None, target_bir_lowering=False)
    x_d = nc.dram_tensor("x_bf", [B, C_IN, T + 2], bf16, kind="ExternalInput")
    wred_d = nc.dram_tensor("wred_t", [3, C_IN, C_HID], bf16, kind="ExternalInput")
    bred_d = nc.dram_tensor("bred", [C_HID, 1], f32, kind="ExternalInput")
    w3d_d = nc.dram_tensor("w3d_t", [N, C_HID, C_ROI], bf16, kind="ExternalInput")
    b3d_d = nc.dram_tensor("b3d", [C_ROI, 1], f32, kind="ExternalInput")
    w2d_d = nc.dram_tensor("w2d_t", [C_ROI, C_OUT], bf16, kind="ExternalInput")
    b2d_d = nc.dram_tensor("b2d", [C_OUT, 1], f32, kind="ExternalInput")
    mask_d = nc.dram_tensor("mask", [T, N * W], bf16, kind="ExternalInput")
    out_d = nc.dram_tensor("out", [B, C_OUT, W], f32, kind="ExternalOutput")

    mask_v = mask_d.rearrange("t (n w) -> t n w", n=N, w=W)

    with tile.TileContext(nc) as tc:
        with (
            tc.tile_pool(name="consts", bufs=1) as consts,
            tc.tile_pool(name="xpool", bufs=1) as xpool,
            tc.tile_pool(name="hpool", bufs=1) as hpool,
            tc.tile_pool(name="w3pool", bufs=1) as w3pool,
            tc.tile_pool(name="ppool", bufs=1) as ppool,
            tc.tile_pool(name="maskpool", bufs=2) as maskpool,
            tc.tile_pool(name="m3pool", bufs=2) as m3pool,
            tc.tile_pool(name="outpool", bufs=4) as outpool,
            tc.tile_pool(name="ps_a", bufs=1, space="PSUM") as ps_a,
            tc.tile_pool(name="ps_b", bufs=2, space="PSUM") as ps_b,
            tc.tile_pool(name="ps_c", bufs=3, space="PSUM") as ps_c,
            tc.tile_pool(name="ps_d", bufs=2, space="PSUM") as ps_d,
        ):
            # ---- small constants + x on the ACT HWDGE ring (keeps the SP
            # ring free for the mask stream).
            wred_sb = consts.tile([128, 6 * C_HID], bf16)
            for k in range(3):
                for u in range(2):
                    nc.scalar.dma_start(
                        wred_sb[:, (k * 2 + u) * C_HID:(k * 2 + u + 1) * C_HID],
                        wred_d[k, u * 128:(u + 1) * 128, :],
                    )
            bred_sb = consts.tile([C_HID, 1], f32)
            nc.scalar.dma_start(bred_sb[:], bred_d[:, :])
            b3d_sb = consts.tile([128, 4], f32)
            for g in range(4):
                nc.scalar.dma_start(b3d_sb[:, g:g + 1], b3d_d[g * 128:(g + 1) * 128, :])
            b2d_sb = consts.tile([C_OUT, 1], f32)
            nc.scalar.dma_start(b2d_sb[:], b2d_d[:, :])
            w2d_sb = consts.tile([128, 4 * C_OUT], bf16)
            for g in range(4):
                nc.scalar.dma_start(
                    w2d_sb[:, g * C_OUT:(g + 1) * C_OUT],
                    w2d_d[g * 128:(g + 1) * 128, :],
                )
            xts = []
            for b in range(B):
                for u in range(2):
                    xt = xpool.tile([128, T + 2], bf16, tag=f"x_{b}_{u}", name=f"x_{b}_{u}")
                    nc.scalar.dma_start(xt[:], x_d[b, u * 128:(u + 1) * 128, :])
                    xts.append(xt)

            # ---- stage A: conv1d + relu -> h
            h_sb = []
            for b in range(B):
                hp = ps_a.tile([C_HID, T], f32, tag="hps", name=f"hps_{b}")
                first = True
                for u in range(2):
                    for k in range(3):
                        nc.tensor.matmul(
                            hp[:],
                            wred_sb[:, (k * 2 + u) * C_HID:(k * 2 + u + 1) * C_HID],
                            xts[b * 2 + u][:, k:k + T],
                            start=first,
                            stop=(u == 1 and k == 2),
                        )
                        first = False
                ht = hpool.tile([C_HID, T], bf16, tag=f"h_{b}", name=f"h_{b}")
                nc.scalar.activation(ht[:], hp[:], Relu, bias=bred_sb[:, 0:1])
                h_sb.append(ht)

            # ---- stage B: P^T[b,n] = [t, o]; w3d streamed per-n so the first
            # B matmuls start as soon as chunk 0 lands.
            P = [[None] * N for _ in range(B)]
            w3_sb = [None] * N
            for n in range(N):
                wt = w3pool.tile([C_HID, C_ROI], bf16, tag=f"w3_{n}", name=f"w3_{n}")
                nc.scalar.dma_start(wt[:], w3d_d[n, :, :])
                w3_sb[n] = wt
            cnt = 0
            for n in range(N):
                for b in range(B):
                    pp = ps_b.tile([T, C_ROI], f32, tag="pps", name=f"pps_{b}_{n}")
                    nc.tensor.matmul(pp[:], h_sb[b][:], w3_sb[n][:], start=True, stop=True)
                    pt = ppool.tile([T, C_ROI], bf16, tag=f"P_{b}_{n}", name=f"P_{b}_{n}")
                    if cnt % 2 == 0:
                        nc.vector.tensor_copy(pt[:], pp[:])
                    else:
                        nc.scalar.copy(pt[:], pp[:])
                    cnt += 1
                    P[b][n] = pt

            # ---- stages C & D per packed-column tile; mask on the SP ring.
            for jt, (c0, cw) in enumerate(COL_TILES):
                mt = maskpool.tile([T, N * cw], bf16, tag="mask", name=f"mask_{jt}")
                nc.sync.dma_start(mt[:], mask_v[:, :, c0:c0 + cw])
                m3 = [[None] * 4 for _ in range(B)]
                for b in range(B):
                    for o4 in range(4):
                        pc = ps_c.tile([128, cw], f32, tag="m3ps", name=f"m3ps_{jt}_{b}_{o4}")
                        for n in range(N):
                            nc.tensor.matmul(
                                pc[:],
                                P[b][n][:, o4 * 128:(o4 + 1) * 128],
                                mt[:, n * cw:(n + 1) * cw],
                                start=(n == 0),
                                stop=(n == N - 1),
                            )
                        m3t = m3pool.tile([128, cw], bf16, tag=f"m3_{b}_{o4}", name=f"m3_{jt}_{b}_{o4}")
                        nc.scalar.activation(m3t[:], pc[:], Relu, bias=b3d_sb[:, o4:o4 + 1])
                        m3[b][o4] = m3t
                for b in range(B):
                    pd = ps_d.tile([C_OUT, cw], f32, tag="outps", name=f"outps_{jt}_{b}")
                    for o4 in range(4):
                        nc.tensor.matmul(
                            pd[:],
                            w2d_sb[:, o4 * C_OUT:(o4 + 1) * C_OUT],
                            m3[b][o4][:],
                            start=(o4 == 0),
                            stop=(o4 == 3),
                        )
                    ot = outpool.tile([C_OUT, cw], f32, tag="out", name=f"out_{jt}_{b}")
                    nc.scalar.activation(ot[:], pd[:], Relu, bias=b2d_sb[:, 0:1])
                    nc.gpsimd.dma_start(out_d[b, :, c0:c0 + cw], ot[:])
    nc.compile()
    return nc


def kernel(**inputs):
    global LAST_EXEC_NS
    x = np.asarray(inputs["x"], dtype=np.float32)
    w_red = np.asarray(inputs["w_red"], dtype=np.float32)
    b_red = np.asarray(inputs["b_red"], dtype=np.float32)
    w3d = np.asarray(inputs["w3d"], dtype=np.float32)
    b3d = np.asarray(inputs["b3d"], dtype=np.float32)
    w2d = np.asarray(inputs["w2d"], dtype=np.float32)
    b2d = np.asarray(inputs["b2d"], dtype=np.float32)
    mask = np.asarray(inputs["sample_mask"], dtype=np.float32)

    x_bf = np.zeros((B, C_IN, T + 2), dtype=BF)
    x_bf[:, :, 1:T + 1] = x.astype(BF)
    wred_t = np.ascontiguousarray(w_red.transpose(2, 1, 0)).astype(BF)   # [3, CI, CH]
    w3d_t = np.ascontiguousarray(w3d.transpose(2, 1, 0)).astype(BF)      # [N, CH, CR]
    w2d_t = np.ascontiguousarray(w2d.transpose(1, 0)).astype(BF)         # [CR, CO]
    bred = np.ascontiguousarray(b_red.reshape(C_HID, 1))
    b3d_r = np.ascontiguousarray(b3d.reshape(C_ROI, 1))
    b2d_r = np.ascontiguousarray(b2d.reshape(C_OUT, 1))
    mask_bf = mask.astype(BF)                                            # [T, N, D, M]

    common = dict(x_bf=x_bf, wred_t=wred_t, bred=bred, w3d_t=w3d_t,
                  b3d=b3d_r, w2d_t=w2d_t, b2d=b2d_r)
    in_maps = []
    dlists = []
    for c in range(NCORES):
        dl = _dlist(c)
        dlists.append(dl)
        mk = np.zeros((T, N, W), dtype=BF)
        col = 0
        for d in dl:
            w = T - d
            mk[:, :, col:col + w] = mask_bf[:, :, d, :w]
            col += w
        in_maps.append(dict(common, mask=mk.reshape(T, N * W)))

    if "nc" not in _CACHE:
        _CACHE["nc"] = _build()
    nc = _CACHE["nc"]

    from concourse.bass_utils import run_bass_kernel_spmd

    trace = os.environ.get("BMN_TRACE", "0") == "1"
    res = run_bass_kernel_spmd(nc, in_maps, core_ids=list(range(NCORES)), trace=trace)
    LAST_EXEC_NS = res.exec_time_ns

    # Invalid (d+m >= T) cells: mask column is zero -> per-channel constant.
    c_m3 = np.maximum(b3d, 0.0)
    c_out = np.maximum(w2d.astype(np.float32) @ c_m3 + b2d, 0.0)         # [C_OUT]
    out = np.empty((B, C_OUT, D, M), dtype=np.float32)
    out[:] = c_out[None, :, None, None]
    for c in range(NCORES):
        res_c = res.results[c]["out"]                                    # [B, C_OUT, W]
        col = 0
        for d in dlists[c]:
            w = T - d
            out[:, :, d, :w] = res_c[:, :, col:col + w]
            col += w
    return out


# revision 13
# speedup vs baseline: 1.6632x; 1.2081x over previous
"""BMN extractor kernel for Trainium2 (8 NeuronCores, Bass/Tile).

Computation (matches the reference nn.Module):
  h   = relu(conv1d(x, w_red, k=3, pad=SAME) + b_red)            [B, CH, T]
  map = einsum('bct,tndm->bcndm', h, mask)                        (never materialized)
  m3  = relu(einsum('ocn,bcndm->bodm', w3d, map) + b3d)           [B, CR, D, M]
  out = relu(einsum('oc,bcdm->bodm', w2d, m3) + b2d)              [B, CO, D, M]

Reassociation used on device:
  P[b,o,n,t]  = sum_c w3d[o,c,n] * h[b,c,t]            (small matmuls)
  m3[b,o,d,m] = sum_{n,t} P[b,o,n,t] * mask[t,n,d,m]   (big matmul, K=N*T=4096)

Cells with d+m >= T have an all-zero mask column, so their output is a
per-channel constant relu(w2d @ relu(b3d) + b2d) — computed host-side.  Only
the 50.4% valid columns are computed on device.  Durations are sharded across
the 8 cores in pairs (d, 127-d) so every core gets exactly 1032 valid
(d,m) columns, padded to W=1088.
"""

import os

import numpy as np
import ml_dtypes

B, C_IN, C_HID, C_ROI, C_OUT = 2, 256, 128, 512, 128
T, N, D, M = 128, 32, 128, 128
NCORES = 8
W = 1088                       # padded packed (d,m) columns per core
COL_TILES = [(0, 512), (512, 512), (1024, 64)]
BF = ml_dtypes.bfloat16

_CACHE = {}
LAST_EXEC_NS = None


def _dlist(core):
    """Duration values handled by `core`: 8 pairs (i, 127-i) -> 1032 valid cols."""
    out = []
    for i in range(core, 64, 8):
        out += [i, 127 - i]
    return out


def _build():
    import concourse.tile as tile
    from concourse import bacc, mybir

    bf16 = mybir.dt.bfloat16
    f32 = mybir.dt.float32
    Relu = mybir.ActivationFunctionType.Relu

    nc = bacc.Bacc(None, target_bir_lowering=False)
    x_d = nc.dram_tensor("x_bf", [B, C_IN, T + 2], bf16, kind="ExternalInput")
    wred_d = nc.dram_tensor("wred_t", [3, C_IN, C_HID], bf16, kind="ExternalInput")
    w3d_d = nc.dram_tensor("w3d_t", [N, C_HID, C_ROI], bf16, kind="ExternalInput")
    w2d_d = nc.dram_tensor("w2d_t", [C_ROI, C_OUT], bf16, kind="ExternalInput")
    bias_d = nc.dram_tensor("biases", [128, 6], f32, kind="ExternalInput")
    mask_d = nc.dram_tensor("mask", [T, N * W], bf16, kind="ExternalInput")
    out_d = nc.dram_tensor("out", [B, C_OUT, W], f32, kind="ExternalOutput")

    mask_v = mask_d.rearrange("t (n w) -> t n w", n=N, w=W)

    with tile.TileContext(nc) as tc:
        with (
            tc.tile_pool(name="consts", bufs=1) as consts,
            tc.tile_pool(name="xpool", bufs=1) as xpool,
            tc.tile_pool(name="hpool", bufs=1) as hpool,
            tc.tile_pool(name="w3pool", bufs=1) as w3pool,
            tc.tile_pool(name="ppool", bufs=1) as ppool,
            tc.tile_pool(name="maskpool", bufs=2) as maskpool,
            tc.tile_pool(name="m3pool", bufs=2) as m3pool,
            tc.tile_pool(name="outpool", bufs=4) as outpool,
            tc.tile_pool(name="ps_a", bufs=1, space="PSUM") as ps_a,
            tc.tile_pool(name="ps_b", bufs=2, space="PSUM") as ps_b,
            tc.tile_pool(name="ps_c", bufs=3, space="PSUM") as ps_c,
            tc.tile_pool(name="ps_d", bufs=2, space="PSUM") as ps_d,
        ):
            # ---- batched constant loads on the ACT HWDGE ring (the SP ring
            # carries only the mask stream).  Small HWDGE DMAs serialize at
            # ~0.65us each, so everything is packed into a few wide DMAs.
            xall = xpool.tile([128, B * 2 * (T + 2)], bf16)
            nc.scalar.dma_start(
                xall[:], x_d.rearrange("b (u p) t -> p b u t", u=2, p=128))
            xts = [xall[:, (b * 2 + u) * (T + 2):(b * 2 + u + 1) * (T + 2)]
                   for b in range(B) for u in range(2)]
            wred_sb = consts.tile([128, 6 * C_HID], bf16)
            nc.scalar.dma_start(
                wred_sb[:], wred_d.rearrange("k (u p) c -> p k u c", u=2, p=128))
            bias_sb = consts.tile([128, 6], f32)
            nc.scalar.dma_start(bias_sb[:], bias_d[:, :])
            bred_sb = bias_sb[:, 0:1]
            b3d_sb = bias_sb[:, 1:5]
            b2d_sb = bias_sb[:, 5:6]
            w2d_sb = consts.tile([128, 4 * C_OUT], bf16)
            nc.scalar.dma_start(
                w2d_sb[:], w2d_d.rearrange("(g p) o -> p g o", g=4, p=128))

            # ---- stage A: conv1d + relu -> h
            h_sb = []
            for b in range(B):
                hp = ps_a.tile([C_HID, T], f32, tag="hps", name=f"hps_{b}")
                first = True
                for u in range(2):
                    for k in range(3):
                        nc.tensor.matmul(
                            hp[:],
                            wred_sb[:, (k * 2 + u) * C_HID:(k * 2 + u + 1) * C_HID],
                            xts[b * 2 + u][:, k:k + T],
                            start=first,
                            stop=(u == 1 and k == 2),
                        )
                        first = False
                ht = hpool.tile([C_HID, T], bf16, tag=f"h_{b}", name=f"h_{b}")
                nc.scalar.activation(ht[:], hp[:], Relu, bias=bred_sb)
                h_sb.append(ht)

            # ---- stage B: P^T[b,n] = [t, o]; w3d streamed in 4 chunks so the
            # first B matmuls start as soon as chunk 0 lands.
            P = [[None] * N for _ in range(B)]
            w3_sb = w3pool.tile([C_HID, N * C_ROI], bf16)
            NG = 8  # n per w3d DMA chunk
            cnt = 0
            for n in range(N):
                if n % NG == 0:
                    nc.scalar.dma_start(
                        w3_sb[:, n * C_ROI:(n + NG) * C_ROI],
                        w3d_d[n:n + NG, :, :].rearrange("n c o -> c n o"),
                    )
                for b in range(B):
                    pp = ps_b.tile([T, C_ROI], f32, tag="pps", name=f"pps_{b}_{n}")
                    nc.tensor.matmul(pp[:], h_sb[b][:],
                                     w3_sb[:, n * C_ROI:(n + 1) * C_ROI],
                                     start=True, stop=True)
                    pt = ppool.tile([T, C_ROI], bf16, tag=f"P_{b}_{n}", name=f"P_{b}_{n}")
                    if cnt % 2 == 0:
                        nc.vector.tensor_copy(pt[:], pp[:])
                    else:
                        nc.scalar.copy(pt[:], pp[:])
                    cnt += 1
                    P[b][n] = pt

            # ---- stages C & D per packed-column tile; mask on the SP ring.
            for jt, (c0, cw) in enumerate(COL_TILES):
                mt = maskpool.tile([T, N * cw], bf16, tag="mask", name=f"mask_{jt}")
                nc.sync.dma_start(mt[:], mask_v[:, :, c0:c0 + cw])
                m3 = [[None] * 4 for _ in range(B)]
                for b in range(B):
                    for o4 in range(4):
                        pc = ps_c.tile([128, cw], f32, tag="m3ps", name=f"m3ps_{jt}_{b}_{o4}")
                        for n in range(N):
                            nc.tensor.matmul(
                                pc[:],
                                P[b][n][:, o4 * 128:(o4 + 1) * 128],
                                mt[:, n * cw:(n + 1) * cw],
                                start=(n == 0),
                                stop=(n == N - 1),
                            )
                        m3t = m3pool.tile([128, cw], bf16, tag=f"m3_{b}_{o4}", name=f"m3_{jt}_{b}_{o4}")
                        nc.scalar.activation(m3t[:], pc[:], Relu, bias=b3d_sb[:, o4:o4 + 1])
                        m3[b][o4] = m3t
                for b in range(B):
                    pd = ps_d.tile([C_OUT, cw], f32, tag="outps", name=f"outps_{jt}_{b}")
                    for o4 in range(4):
                        nc.tensor.matmul(
                            pd[:],
                            w2d_sb[:, o4 * C_OUT:(o4 + 1) * C_OUT],
                            m3[b][o4][:],
                            start=(o4 == 0),
                            stop=(o4 == 3),
                        )
                    ot = outpool.tile([C_OUT, cw], f32, tag="out", name=f"out_{jt}_{b}")
                    nc.scalar.activation(ot[:], pd[:], Relu, bias=b2d_sb)
                    nc.gpsimd.dma_start(out_d[b, :, c0:c0 + cw], ot[:])
    nc.compile()
    return nc


def kernel(**inputs):
    global LAST_EXEC_NS
    x = np.asarray(inputs["x"], dtype=np.float32)
    w_red = np.asarray(inputs["w_red"], dtype=np.float32)
    b_red = np.asarray(inputs["b_red"], dtype=np.float32)
    w3d = np.asarray(inputs["w3d"], dtype=np.float32)
    b3d = np.asarray(inputs["b3d"], dtype=np.float32)
    w2d = np.asarray(inputs["w2d"], dtype=np.float32)
    b2d = np.asarray(inputs["b2d"], dtype=np.float32)
    mask = np.asarray(inputs["sample_mask"], dtype=np.float32)

    x_bf = np.zeros((B, C_IN, T + 2), dtype=BF)
    x_bf[:, :, 1:T + 1] = x.astype(BF)
    wred_t = np.ascontiguousarray(w_red.transpose(2, 1, 0)).astype(BF)   # [3, CI, CH]
    w3d_t = np.ascontiguousarray(w3d.transpose(2, 1, 0)).astype(BF)      # [N, CH, CR]
    w2d_t = np.ascontiguousarray(w2d.transpose(1, 0)).astype(BF)         # [CR, CO]
    biases = np.stack([b_red, b3d[0:128], b3d[128:256], b3d[256:384],
                       b3d[384:512], b2d], axis=1).astype(np.float32)    # [128, 6]
    biases = np.ascontiguousarray(biases)
    mask_bf = mask.astype(BF)                                            # [T, N, D, M]

    common = dict(x_bf=x_bf, wred_t=wred_t, w3d_t=w3d_t,
                  w2d_t=w2d_t, biases=biases)
    in_maps = []
    dlists = []
    for c in range(NCORES):
        dl = _dlist(c)
        dlists.append(dl)
        mk = np.zeros((T, N, W), dtype=BF)
        col = 0
        for d in dl:
            w = T - d
            mk[:, :, col:col + w] = mask_bf[:, :, d, :w]
            col += w
        in_maps.append(dict(common, mask=mk.reshape(T, N * W)))

    if "nc" not in _CACHE:
        _CACHE["nc"] = _build()
    nc = _CACHE["nc"]

    from concourse.bass_utils import run_bass_kernel_spmd

    trace = os.environ.get("BMN_TRACE", "0") == "1"
    res = run_bass_kernel_spmd(nc, in_maps, core_ids=list(range(NCORES)), trace=trace)
    LAST_EXEC_NS = res.exec_time_ns

    # Invalid (d+m >= T) cells: mask column is zero -> per-channel constant.
    c_m3 = np.maximum(b3d, 0.0)
    c_out = np.maximum(w2d.astype(np.float32) @ c_m3 + b2d, 0.0)         # [C_OUT]
    out = np.empty((B, C_OUT, D, M), dtype=np.float32)
    out[:] = c_out[None, :, None, None]
    for c in range(NCORES):
        res_c = res.results[c]["out"]                                    # [B, C_OUT, W]
        col = 0
        for d in dlists[c]:
            w = T - d
            out[:, :, d, :w] = res_c[:, :, col:col + w]
            col += w
    return out
